# revision 1
# baseline (speedup 1.0000x reference)
"""DynamicConv1D Trainium2 kernel (three-engine conv pipeline).

Reference computation (per batch b, position s):
    kern[s, h, i] = sum_c x[s, c] * W_pred[c, h*7+i] + b_pred[h*7+i]
    out[s, h, d]  = sum_i kern[s, h, i] * x_pad[s + i, h, d]     (pad = 3)

Sharding: 8 cores = (batch 4) x (sequence halves 2). Each core receives the
transposed shard xT [C=1024, 1030] f32 (1024 positions + 3-halo each side,
zero-padded at sequence ends) and produces outT [1024, 1024] bf16 (host
upcasts to f32 in assemble()).

Per-core pipeline:
  1. kern matmul on PE (fp32r, exact-ish): lhsT = W chunk [128c, 112],
     rhs = xT chunk [128c, 512] -> PSUM [112, 512] accumulated over 8
     c-chunks.  ACT copies PSUM -> SBUF with per-partition f32 bias.
  2. kb broadcast on PE: selection matmul with constant 0/1 sel[:, t, i, :]
     [112, 128] -> kb pair tiles [128, 2, 512] in PSUM (fp32).
  3. Conv in 16 mini-tiles (channel tile t x column half h), tap products
     split across engines (GPSIMD cannot touch PSUM; only DVE/ACT drain it):
       taps 0,1: DVE multiplies the PSUM pair directly (f32 in, bf16 out).
       taps 2..6: ACT casts kb PSUM -> SBUF f32; Pool (gpsimd) multiplies
                  (f32 in, bf16 out).
     Add tree: DVE W[0:2]=P01+P23 (bf16 2x); DVE strided U=(W0+W1, W2+W3);
     DVE Dt=U1+P6; Pool fin=U0+Dt.  kb PSUM groups are [128, 2, 512]
     (2 banks) x bufs=4 so PE fills run a full mini-tile ahead of the
     consumers (a [2,1024]x2 layout serializes PE against DVE/ACT).
"""

import os
import sys

for _p in ("/opt/trn_rl_repo",):
    if _p not in sys.path and os.path.isdir(_p):
        sys.path.insert(0, _p)

import numpy as np
import ml_dtypes

import concourse.bass as bass
import concourse.mybir as mybir
from concourse import tile
from concourse.bass_utils import run_bass_kernel_spmd
from concourse.bass_types import AP

B = 4
S = 2048
C = 1024
H = 16
K = 7
HD = 64
PAD = K // 2
KH = K * H  # 112

N_CORES = 8
SHARD = S // 2          # positions per core = 1024
SH = SHARD + 2 * PAD    # 1030 source positions incl halo
NT = C // 128           # 8 channel tiles
SW = 512                # free-dim tile (one PSUM bank of fp32)
NS = SHARD // SW        # 2 position tiles

F32 = mybir.dt.float32
F32R = mybir.dt.float32r
BF16 = mybir.dt.bfloat16


def _build_sel() -> np.ndarray:
    """sel[p, t, i, c] = 1 iff p == (2t + c//64)*7 + i  (p in [0,112))."""
    sel = np.zeros((KH, NT, K, 128), dtype=np.float32)
    for t in range(NT):
        for i in range(K):
            for hh in range(2):  # two heads per 128-channel tile
                p = (2 * t + hh) * K + i
                sel[p, t, i, 64 * hh:64 * (hh + 1)] = 1.0
    return sel


def build_program() -> bass.Bass:
    nc = bass.Bass(trn_type="TRN2")

    xt_d = nc.dram_tensor("xt", [C, SH], F32R, kind="ExternalInput")
    wb_d = nc.dram_tensor("wb", [128, NT * KH + 1], F32R, kind="ExternalInput")
    sel_d = nc.dram_tensor("sel", [KH, NT * K * 128], BF16, kind="ExternalInput")
    out_d = nc.dram_tensor("out", [C, SHARD], BF16, kind="ExternalOutput")

    with tile.TileContext(nc) as tc:
        with (
            tc.tile_pool(name="xt", bufs=1) as xt_pool,
            tc.tile_pool(name="wgt", bufs=1) as w_pool,
            tc.tile_pool(name="sel", bufs=1) as sel_pool,
            tc.tile_pool(name="kern", bufs=1) as kern_pool,
            tc.tile_pool(name="fin", bufs=2) as fin_pool,
            tc.tile_pool(name="prime", bufs=1) as prime_pool,
        ):
            # ---- loads ----
            # Each input completes on a single DMA sem lane; primers below
            # let every later instruction carry at most one sem wait.
            SPLIT = PAD + SW + PAD  # 518
            xt_sb = xt_pool.tile([128, NT, SH], F32R)
            xt_r = xt_d.ap().rearrange("(t p) s -> p t s", p=128)
            wb_sb = w_pool.tile([128, NT * KH + 1], F32R)
            # Everything rides the SP ring, interleaved so each chunk lands
            # just before its consumer needs it (kern sj0 -> blocks,
            # conv tile t -> sel[t], kern sj1 -> rem pairs).  This keeps
            # the big DMA cost off the ACT/Pool/DVE engine timelines.
            sel_sb = sel_pool.tile([KH, NT, K, 128], BF16)
            sel_r = sel_d.ap().rearrange("p (t x) -> p t x", t=NT)

            def sel_dma(t0, t1):
                nc.sync.dma_start(
                    sel_sb[:, t0:t1, :, :].rearrange("p t i c -> p (t i c)"),
                    sel_r[:, t0:t1, :].rearrange("p t x -> p (t x)"),
                )

            def xt_block(q):
                nc.sync.dma_start(xt_sb[:, 2 * q:2 * q + 2, 0:SPLIT],
                                  xt_r[:, 2 * q:2 * q + 2, 0:SPLIT])

            def xt_rem(m):
                nc.sync.dma_start(xt_sb[:, 2 * m:2 * m + 2, SPLIT:SH],
                                  xt_r[:, 2 * m:2 * m + 2, SPLIT:SH])

            nc.sync.dma_start(wb_sb[:, :], wb_d[:, :])
            for q in range(4):
                xt_block(q)
            sel_dma(0, 1)
            sel_dma(1, 2)
            xt_rem(0)
            xt_rem(1)
            sel_dma(2, 4)
            xt_rem(2)
            xt_rem(3)
            sel_dma(4, 6)
            sel_dma(6, 8)

            FW = NS * SW  # 1024
            kern_sb = kern_pool.tile([KH, SHARD], BF16)

            # All PSUM (8 banks) belongs to one pool of [128, 2, 512]
            # (2-bank) tiles; kern-phase accumulators and primer scratch
            # allocate from it too, so conv fills always have 4 groups
            # (one full mini-tile) in flight.
            with (
                tc.tile_pool(name="kb", bufs=4, space="PSUM") as kb_pool,
                tc.tile_pool(name="cast", bufs=3) as cast_pool,
                tc.tile_pool(name="prod", bufs=3) as prod_pool,
                tc.tile_pool(name="acc", bufs=3) as acc_pool,
            ):
                # ---- semaphore-lane primers ----
                # All inputs ride the SP lane with monotone counts, so after
                # one observation per engine every later instruction needs at
                # most one sem wait (NoOp-splitting covers stragglers).
                tiny = prime_pool.tile([1, 12], F32)
                tiny_ps = kb_pool.tile([128, 2, SW], F32, tag="kb")
                nc.tensor.matmul(tiny_ps[0:2, 0, 0:2], wb_sb[0:2, 0:2],
                                 wb_sb[0:2, 0:2], start=True, stop=True)
                nc.tensor.matmul(tiny_ps[0:2, 0, 2:4], xt_sb[0:2, 0, 0:2],
                                 xt_sb[0:2, 0, 0:2], start=True, stop=True)
                nc.scalar.copy(tiny[:, 0:1],
                               wb_sb[0:1, NT * KH:NT * KH + 1].bitcast(F32))
                nc.vector.tensor_copy(tiny[:, 1:2],
                                      xt_sb[0:1, 0, 0:1].bitcast(F32))
                nc.gpsimd.tensor_copy(tiny[:, 8:9],
                                      xt_sb[0:1, 0, 0:1].bitcast(F32))

                def kern_phase(sj):
                    kps = kb_pool.tile([128, 2, SW], F32, tag="kb")
                    for m in range(NT):
                        nc.tensor.matmul(
                            kps[0:KH, 0, :], wb_sb[:, KH * m:KH * (m + 1)],
                            xt_sb[:, m, PAD + SW * sj:PAD + SW * sj + SW],
                            start=(m == 0), stop=(m == NT - 1),
                        )
                    nc.scalar.activation(
                        kern_sb[:, SW * sj:SW * (sj + 1)], kps[0:KH, 0, :],
                        mybir.ActivationFunctionType.Identity,
                        bias=wb_sb[0:KH, NT * KH:NT * KH + 1].bitcast(F32),
                    )

                def xt_pair(t, off):
                    """[128, 2, 512] view: (a, s) -> xt[:, t, off + a + s]."""
                    base = xt_sb[:, t, off:off + SW]
                    return AP(tensor=base.tensor, offset=base.offset,
                              ap=[[base.ap[0][0], 128], [1, 2],
                                  [1, SW]]).bitcast(F32)

                def mini(t, h):
                    s0 = SW * h
                    # -- PE fills: 4 groups of [128, 2, 512] (g3 half) --
                    kbs = []
                    for g in range(4):
                        kb = kb_pool.tile([128, 2, SW], F32, tag="kb")
                        npair = 2 if g < 3 else 1
                        for a in range(npair):
                            i = 2 * g + a
                            nc.tensor.matmul(
                                kb[:, a, :], sel_sb[:, t, i, :],
                                kern_sb[:, s0:s0 + SW],
                                start=True, stop=True,
                            )
                        kbs.append(kb)
                    # -- products --
                    P01 = prod_pool.tile([128, 2, SW], BF16, tag="p01")
                    nc.vector.tensor_mul(P01[:, :, :], kbs[0][:, :, :],
                                         xt_pair(t, s0))
                    c23 = cast_pool.tile([128, 2, SW], F32, tag="c23")
                    nc.scalar.activation(
                        c23[:, :, :], kbs[1][:, :, :],
                        mybir.ActivationFunctionType.Identity)
                    P23 = prod_pool.tile([128, 2, SW], BF16, tag="p23")
                    nc.gpsimd.tensor_mul(P23[:, :, :], c23[:, :, :],
                                         xt_pair(t, s0 + 2))
                    W4 = acc_pool.tile([128, 4, SW], BF16, tag="w4")
                    c45 = cast_pool.tile([128, 2, SW], F32, tag="c45")
                    nc.scalar.activation(
                        c45[:, :, :], kbs[2][:, :, :],
                        mybir.ActivationFunctionType.Identity)
                    nc.gpsimd.tensor_mul(W4[:, 2:4, :], c45[:, :, :],
                                         xt_pair(t, s0 + 4))
                    c6 = cast_pool.tile([128, SW], F32, tag="c6")
                    nc.scalar.activation(
                        c6[:, :], kbs[3][:, 0, :],
                        mybir.ActivationFunctionType.Identity)
                    P6 = prod_pool.tile([128, SW], BF16, tag="p6")
                    nc.gpsimd.tensor_mul(
                        P6[:, :], c6[:, :],
                        xt_sb[:, t, s0 + 6:s0 + 6 + SW].bitcast(F32))
                    # -- add tree --
                    nc.vector.tensor_add(W4[:, 0:2, :], P01[:, :, :],
                                         P23[:, :, :])
                    U = acc_pool.tile([128, 2, SW], BF16, tag="u")
                    wev = AP(tensor=W4[:, 0, :].tensor,
                             offset=W4[:, 0, :].offset,
                             ap=[[W4[:, 0, :].ap[0][0], 128],
                                 [2 * SW, 2], [1, SW]])
                    wod = AP(tensor=W4[:, 1, :].tensor,
                             offset=W4[:, 1, :].offset,
                             ap=[[W4[:, 1, :].ap[0][0], 128],
                                 [2 * SW, 2], [1, SW]])
                    nc.vector.tensor_add(U[:, :, :], wev, wod)
                    Dt = acc_pool.tile([128, SW], BF16, tag="dt")
                    nc.vector.tensor_add(Dt[:, :], U[:, 1, :], P6[:, :])
                    fin = fin_pool.tile([128, SW], BF16, tag="fin")
                    nc.gpsimd.tensor_add(fin[:, :], U[:, 0, :], Dt[:, :])
                    nc.sync.dma_start(out_d[128 * t:128 * (t + 1),
                                            s0:s0 + SW],
                                      fin[:, :])

                # kern sj1 is emitted after two h=0 mini-tiles: its matmuls
                # wait on the xt remainder DMAs, and PE dispatches in program
                # order -- the first minis keep all engines busy meanwhile.
                kern_phase(0)
                mini(0, 0)
                mini(1, 0)
                kern_phase(1)
                for t in range(2, NT):
                    mini(t, 0)
                for t in range(NT):
                    mini(t, 1)

    _strip_same_engine_waits(nc)
    return nc


# Engines complete their own instructions in program order (PE matmuls are
# pc-monotone in start AND end), so a wait on the engine's own completion
# semaphore is always satisfied by program order.  Tile still emits them for
# PSUM-slot WAW tracking; walrus then rejects matmuls with >1 wait (the
# LDWEIGHTS struct has a single sync-wait slot).  Strip them.
def _strip_same_engine_waits(nc: bass.Bass) -> None:
    # (1) PE matmuls complete in strict pc order (silicon), so a wait on PE's
    # own completion semaphore is redundant -> strip, keeping each matmul at
    # <=1 wait (walrus LDWEIGHTS has a single sync-wait slot).
    # (2) The exit Drain waits on every semaphore ever used, exceeding the
    # struct's wait capacity.  Input-DMA lane waits are covered transitively:
    # each compute engine waited on those lanes before its last instruction,
    # and the Drain still waits on every engine's final count.  Keep only
    # engine sems and the out-DMA lanes (nothing else observes those).
    out_lanes = set()
    for blk in nc.m.functions[0].blocks:
        for inst in blk.instructions:
            if inst.opcode != "DMACopy":
                continue
            dst = inst.outs[0]
            if getattr(dst, "memref", "").startswith("out"):
                for u in (inst.sync_info.on_update if inst.sync_info else []):
                    out_lanes.add(u.ant_name)
    noop_n = [0]
    for blk in nc.m.functions[0].blocks:
        for inst in blk.instructions:
            si = inst.sync_info
            if si is None or not si.on_wait:
                continue
            if str(inst.engine) == "EngineType.PE":
                kept = [w for w in si.on_wait if not w.ant_name.startswith("PE_")]
            elif inst.opcode == "Drain":
                kept = [
                    w for w in si.on_wait
                    if not w.ant_name.startswith("DMAHW") or w.ant_name in out_lanes
                ]
            else:
                continue
            if len(kept) != len(si.on_wait):
                inst.sync_info = mybir.SyncInfo(
                    on_wait=kept, on_update=list(si.on_update)
                )
    # Any instruction still carrying >1 wait: keep the first wait and move the
    # extras onto single-wait NoOps inserted just before it (same engine) --
    # the walrus instruction structs have a single sync-wait slot.
    for blk in nc.m.functions[0].blocks:
        il = blk.instructions
        idx = 0
        while idx < len(il):
            inst = il[idx]
            si = inst.sync_info
            if si is not None and len(si.on_wait) > 1:
                waits = list(si.on_wait)
                for w in waits[:-1]:
                    noop_n[0] += 1
                    nop = mybir.InstNoOp(
                        name=f"I-waitsplit-{noop_n[0]}",
                        engine=inst.engine,
                        ins=[], outs=[],
                        sync_info=mybir.SyncInfo(on_wait=[w], on_update=[]),
                    )
                    nc.register_instruction(nop, overwrite=True)
                    il.insert(idx, nop)
                    idx += 1
                inst.sync_info = mybir.SyncInfo(
                    on_wait=[waits[-1]], on_update=list(si.on_update)
                )
            idx += 1


_PROGRAM = None


def _get_program() -> bass.Bass:
    global _PROGRAM
    if _PROGRAM is None:
        _PROGRAM = build_program()
    return _PROGRAM


def make_in_maps(x: np.ndarray, W_pred: np.ndarray, b_pred: np.ndarray):
    sel = np.ascontiguousarray(
        _build_sel().reshape(KH, NT * K * 128).astype(ml_dtypes.bfloat16))
    # wb blob: [p, t*KH + k] = W_pred[t*128 + p, k]; last column = b_pred
    wb = np.zeros((128, NT * KH + 1), dtype=np.float32)
    wb[:, :NT * KH] = (
        np.asarray(W_pred, dtype=np.float32)
        .reshape(NT, 128, KH).transpose(1, 0, 2).reshape(128, NT * KH)
    )
    wb[:KH, NT * KH] = np.asarray(b_pred, dtype=np.float32)
    in_maps = []
    for core in range(N_CORES):
        b_idx, half = divmod(core, 2)
        s0 = half * SHARD
        xp = np.zeros((SH, C), dtype=np.float32)
        lo = max(0, s0 - PAD)
        hi = min(S, s0 + SHARD + PAD)
        xp[lo - (s0 - PAD):hi - (s0 - PAD)] = x[b_idx, lo:hi]
        xt = np.ascontiguousarray(xp.T)
        in_maps.append({"xt": xt, "wb": wb, "sel": sel})
    return in_maps


def assemble(results) -> np.ndarray:
    out = np.empty((B, S, C), dtype=np.float32)
    for core in range(N_CORES):
        b_idx, half = divmod(core, 2)
        out[b_idx, half * SHARD:(half + 1) * SHARD] = (
            results[core]["out"].astype(np.float32).T
        )
    return out


def kernel(x: np.ndarray, W_pred: np.ndarray, b_pred: np.ndarray) -> np.ndarray:
    nc = _get_program()
    in_maps = make_in_maps(np.asarray(x), np.asarray(W_pred), np.asarray(b_pred))
    res = run_bass_kernel_spmd(nc, in_maps, list(range(N_CORES)))
    return assemble(res.results)



# revision 2
# speedup vs baseline: 1.0569x; 1.0569x over previous
"""DynamicConv1D Trainium2 kernel v2 (drain/direct split conv pipeline).

Reference computation (per batch b, position s):
    kern[s, h, i] = sum_c x[s, c] * W_pred[c, h*7+i] + b_pred[h*7+i]
    out[s, h, d]  = sum_i kern[s, h, i] * x_pad[s + i, h, d]     (pad = 3)

Sharding: 8 cores = (batch 4) x (sequence halves 2).  Each core gets the
transposed bf16 shard xt [1024, 1030] (1024 positions + 3-halo, zero-padded
at sequence ends) and produces out [1024c, 1024s] bf16.

Per-core pipeline (bf16 everywhere except PSUM):
  kern:  PE matmul (bf16 in, f32 PSUM) -> ACT drain (+bias) -> bf16 SBUF.
  conv per (ctile t, half h):  kb[c,s] = kern[head(c)*7+i, s] per tap i
    PE broadcast-fills kb taps into f32 PSUM pair tiles [128, 2, 512].
    taps 0-3: ACT drains pairs -> SBUF bf16; Pool multiplies vs shifted xt
              (gpsimd has no alignment-sensitive fast mode to lose).
    taps 4-6: DVE multiplies f32 PSUM directly (1x mode, saves the drain).
    tap-sum:  two SWDGE CCE accumulate-DMAs fold p0..p3 in place; the
              remaining three joins alternate DVE/Pool via FLEX.
  out: per-ctile DMA [128, 1024] bf16 -> HBM.
"""

import os
import sys

for _p in ("/opt/trn_rl_repo",):
    if _p not in sys.path and os.path.isdir(_p):
        sys.path.insert(0, _p)

import numpy as np
import ml_dtypes

import concourse.bass as bass
import concourse.mybir as mybir
from concourse import tile
from concourse.bass_utils import run_bass_kernel_spmd
from concourse.bass_types import AP

B = 4
S = 2048
C = 1024
H = 16
K = 7
HD = 64
PAD = K // 2
KH = K * H  # 112

N_CORES = 8
SHARD = S // 2          # positions per core = 1024
SH = SHARD + 2 * PAD    # 1030 cols of xt (position s lives at col s+3)
NT = C // 128           # 8 channel tiles
SW = 512                # free-dim tile (one PSUM bank of f32)
NS = SHARD // SW        # 2 position halves

F32 = mybir.dt.float32
BF16 = mybir.dt.bfloat16
IDENT = mybir.ActivationFunctionType.Identity
ADD = mybir.AluOpType.add


def _build_sel() -> np.ndarray:
    """sel[p, t, i, c] = 1 iff p == (2t + c//64)*7 + i  (p in [0,112))."""
    sel = np.zeros((KH, NT, K, 128), dtype=np.float32)
    for t in range(NT):
        for i in range(K):
            for hh in range(2):  # two heads per 128-channel tile
                p = (2 * t + hh) * K + i
                sel[p, t, i, 64 * hh:64 * (hh + 1)] = 1.0
    return sel


def _shift_view(base: AP, n: int, sw: int) -> AP:
    """[128, n, sw] view: (j, s) -> base[:, j + s] (consecutive tap shifts)."""
    return AP(tensor=base.tensor, offset=base.offset,
              ap=[[base.ap[0][0], 128], [1, n], [1, sw]])


def build_program() -> bass.Bass:
    nc = bass.Bass(trn_type="TRN2")

    xt_d = nc.dram_tensor("xt", [C, SH], BF16, kind="ExternalInput")
    wb_d = nc.dram_tensor("wb", [128, NT * KH], BF16, kind="ExternalInput")
    bias_d = nc.dram_tensor("bias", [KH, 1], F32, kind="ExternalInput")
    sel_d = nc.dram_tensor("sel", [KH, NT * K * 128], BF16, kind="ExternalInput")
    id_d = nc.dram_tensor("ident", [128, 128], BF16, kind="ExternalInput")
    out_d = nc.dram_tensor("out", [C, SHARD], BF16, kind="ExternalOutput")

    with tile.TileContext(nc) as tc:
        with (
            tc.tile_pool(name="xt", bufs=1) as xt_pool,
            tc.tile_pool(name="wgt", bufs=1) as w_pool,
            tc.tile_pool(name="sel", bufs=1) as sel_pool,
            tc.tile_pool(name="kern", bufs=1) as kern_pool,
            tc.tile_pool(name="kbd", bufs=3) as kbd_pool,
            tc.tile_pool(name="prod", bufs=3) as prod_pool,
            tc.tile_pool(name="stage", bufs=NT) as stage_pool,
        ):
            # ---- loads ----
            xt_sb = xt_pool.tile([128, NT, SH], BF16)
            xt_r = xt_d.ap().rearrange("(t p) s -> p t s", p=128)
            wb_sb = w_pool.tile([128, NT, KH], BF16)
            wb_r = wb_d.ap().rearrange("p (t k) -> p t k", t=NT)
            bias_sb = w_pool.tile([KH, 1], F32)
            id_sb = w_pool.tile([128, 128], BF16)
            sel_sb = sel_pool.tile([KH, NT, K, 128], BF16)
            sel_r = sel_d.ap().rearrange("p (t x) -> p t x", t=NT)

            SPL = PAD + SW + PAD + 2  # 520: cols for kern sj0 + conv h0
            nc.sync.dma_start(wb_sb[:, :, :].rearrange("p t k -> p (t k)"),
                              wb_r[:, :, :].rearrange("p t k -> p (t k)"))
            # left columns (kern sj0 + conv h0), fine chunks so the kern
            # matmul chain pipelines behind the loads
            for q0 in (0, 2, 4, 6):
                nc.sync.dma_start(xt_sb[:, q0:q0 + 2, 0:SPL],
                                  xt_r[:, q0:q0 + 2, 0:SPL])
            nc.sync.dma_start(bias_sb[:, :], bias_d[:, :])

            def sel_dma(t0, t1):
                nc.sync.dma_start(
                    sel_sb[:, t0:t1, :, :].rearrange("p t i c -> p (t i c)"),
                    sel_r[:, t0:t1, :].rearrange("p t x -> p (t x)"),
                )

            sel_dma(0, 1)
            nc.sync.dma_start(id_sb[:, :], id_d[:, :])
            sel_dma(1, 2)
            for q0 in (0, 4):
                nc.sync.dma_start(xt_sb[:, q0:q0 + 4, SPL:SH],
                                  xt_r[:, q0:q0 + 4, SPL:SH])
            sel_dma(2, 5)
            sel_dma(5, 8)

            kern_sb = kern_pool.tile([KH, SHARD], BF16)
            stages = []
            for _t in range(NT):
                stage_t = stage_pool.tile([128, NS, SW], BF16, tag="stage",
                                          name=f"stage_{_t}")
                stages.append(stage_t)

            # ---- conv ----
            with (
                tc.tile_pool(name="kb", bufs=3, space="PSUM") as kb_pool,
                tc.tile_pool(name="kb6p", bufs=1, space="PSUM") as kb6_pool,
                tc.tile_pool(name="accp", bufs=1, space="PSUM") as acc_pool,
            ):
                prev = []

                def kern_phase(sj):
                    kps = acc_pool.tile([128, SW], F32, tag="acc",
                                        name=f"kps_{sj}")
                    if sj == 0:
                        # load the ACT function table off the critical path
                        nc.scalar.activation(kern_sb[0:2, 0:2],
                                             wb_sb[0:2, 0, 0:2], IDENT)
                        # PE p-state warm-up: ~3us of tiny matmuls so the
                        # kern/fill stream runs at full clock (overwritten
                        # by the start=True kern matmul below).
                        for _w in range(16):
                            nc.tensor.matmul(kps[0:2, 0:2], wb_sb[0:2, 0, 0:2],
                                             wb_sb[0:2, 0, 0:2],
                                             start=True, stop=True)
                    for m in range(NT):
                        nc.tensor.matmul(
                            kps[0:KH, :], wb_sb[:, m, :],
                            xt_sb[:, m, PAD + SW * sj:PAD + SW * sj + SW],
                            start=(m == 0), stop=(m == NT - 1),
                        )
                    nc.scalar.activation(
                        kern_sb[:, SW * sj:SW * (sj + 1)], kps[0:KH, :],
                        IDENT, bias=bias_sb[:, :],
                    )

                def acc_flush(pv):
                    # PE accumulate (one half behind) + DVE final join
                    t, h, At, p45t, p6t = pv
                    acc = acc_pool.tile([128, SW], F32, tag="acc")
                    srcs = (At[:, 0, :], At[:, 1, :],
                            p45t[:, 0, :], p45t[:, 1, :])
                    for n, src in enumerate(srcs):
                        nc.tensor.matmul(acc[:, :], id_sb[:, :], src,
                                         start=(n == 0), stop=(n == 3))
                    st = stages[t]
                    nc.vector.tensor_add(st[:, h, :], acc[:, :], p6t[:, :])
                    if h == NS - 1:
                        nc.sync.dma_start(
                            out_d[128 * t:128 * (t + 1), :],
                            st[:, :, :].rearrange("p n s -> p (n s)"))

                def half(t, h):
                    s0 = SW * h
                    # -- PE kb fills: pairs (4,5) (0,1) (2,3) + single 6 --
                    kb45 = kb_pool.tile([128, 2, SW], F32, tag="kb")
                    kb01 = kb_pool.tile([128, 2, SW], F32, tag="kb")
                    kb23 = kb_pool.tile([128, 2, SW], F32, tag="kb")
                    kb6 = kb6_pool.tile([128, SW], F32, tag="kb6")
                    for a, kb in ((0, kb01), (4, kb45), (2, kb23)):
                        for j in range(2):
                            nc.tensor.matmul(
                                kb[:, j, :], sel_sb[:, t, a + j, :],
                                kern_sb[:, s0:s0 + SW],
                                start=True, stop=True,
                            )
                    nc.tensor.matmul(kb6[:, :], sel_sb[:, t, 6, :],
                                     kern_sb[:, s0:s0 + SW],
                                     start=True, stop=True)
                    # -- taps 4-6: DVE direct from f32 PSUM --
                    p45 = prod_pool.tile([128, 2, SW], BF16, tag="p45")
                    nc.vector.tensor_mul(
                        p45[:, :, :], kb45[:, :, :],
                        _shift_view(xt_sb[:, t, s0 + 4:s0 + SW + 5], 2, SW))
                    p6 = prod_pool.tile([128, SW], BF16, tag="p6")
                    nc.vector.tensor_mul(
                        p6[:, :], kb6[:, :],
                        xt_sb[:, t, s0 + 6:s0 + 6 + SW])
                    # -- taps 0-3: ACT drains -> Pool product + fold --
                    kbd = kbd_pool.tile([128, 4, SW], BF16, tag="kbd")
                    nc.scalar.activation(kbd[:, 0:2, :], kb01[:, :, :], IDENT)
                    nc.scalar.activation(kbd[:, 2:4, :], kb23[:, :, :], IDENT)
                    p03 = prod_pool.tile([128, 4, SW], BF16, tag="p03")
                    nc.gpsimd.tensor_mul(
                        p03[:, :, :], kbd[:, :, :],
                        _shift_view(xt_sb[:, t, s0:s0 + SW + 3], 4, SW))
                    At = prod_pool.tile([128, 2, SW], BF16, tag="at")
                    nc.gpsimd.tensor_add(At[:, :, :], p03[:, 0:2, :],
                                         p03[:, 2:4, :])
                    # accumulate + final join, two halves behind (keeps
                    # PE fills from stalling on DVE/Pool of the last half)
                    prev.append((t, h, At, p45, p6))
                    if len(prev) > 2:
                        acc_flush(prev.pop(0))

                kern_phase(0)
                half(0, 0)
                half(1, 0)
                kern_phase(1)
                for t in range(2, NT):
                    half(t, 0)
                for t in range(NT - 1):
                    half(t, 1)
                acc_flush(prev.pop(0))
                half(NT - 1, 1)
                for pv in prev:
                    acc_flush(pv)

    _strip_same_engine_waits(nc)
    return nc


# Engines complete their own instructions in program order (PE matmuls are
# pc-monotone in start AND end), so a wait on the engine's own completion
# semaphore is always satisfied by program order.  Tile still emits them for
# PSUM-slot WAW tracking; walrus then rejects matmuls with >1 wait (the
# LDWEIGHTS struct has a single sync-wait slot).  Strip them.
def _strip_same_engine_waits(nc: bass.Bass) -> None:
    # (1) PE matmuls complete in strict pc order (silicon), so a wait on PE's
    # own completion semaphore is redundant -> strip, keeping each matmul at
    # <=1 wait (walrus LDWEIGHTS has a single sync-wait slot).
    # (2) The exit Drain waits on every semaphore ever used, exceeding the
    # struct's wait capacity.  Input-DMA lane waits are covered transitively:
    # each compute engine waited on those lanes before its last instruction,
    # and the Drain still waits on every engine's final count.  Keep only
    # engine sems and the out-DMA lanes (nothing else observes those).
    out_lanes = set()
    for blk in nc.m.functions[0].blocks:
        for inst in blk.instructions:
            if inst.opcode != "DMACopy":
                continue
            dst = inst.outs[0]
            if getattr(dst, "memref", "").startswith("out"):
                for u in (inst.sync_info.on_update if inst.sync_info else []):
                    out_lanes.add(u.ant_name)
    noop_n = [0]
    for blk in nc.m.functions[0].blocks:
        for inst in blk.instructions:
            si = inst.sync_info
            if si is None or not si.on_wait:
                continue
            if str(inst.engine) == "EngineType.PE":
                kept = [w for w in si.on_wait if not w.ant_name.startswith("PE_")]
            elif inst.opcode == "Drain":
                kept = [
                    w for w in si.on_wait
                    if not w.ant_name.startswith("DMAHW") or w.ant_name in out_lanes
                ]
            else:
                continue
            if len(kept) != len(si.on_wait):
                inst.sync_info = mybir.SyncInfo(
                    on_wait=kept, on_update=list(si.on_update)
                )
    # Any instruction still carrying >1 wait: keep the first wait and move the
    # extras onto single-wait NoOps inserted just before it (same engine) --
    # the walrus instruction structs have a single sync-wait slot.
    for blk in nc.m.functions[0].blocks:
        il = blk.instructions
        idx = 0
        while idx < len(il):
            inst = il[idx]
            si = inst.sync_info
            if si is not None and len(si.on_wait) > 1:
                waits = list(si.on_wait)
                for w in waits[:-1]:
                    noop_n[0] += 1
                    nop = mybir.InstNoOp(
                        name=f"I-waitsplit-{noop_n[0]}",
                        engine=inst.engine,
                        ins=[], outs=[],
                        sync_info=mybir.SyncInfo(on_wait=[w], on_update=[]),
                    )
                    nc.register_instruction(nop, overwrite=True)
                    il.insert(idx, nop)
                    idx += 1
                inst.sync_info = mybir.SyncInfo(
                    on_wait=[waits[-1]], on_update=list(si.on_update)
                )
            idx += 1



_PROGRAM = None


def _get_program() -> bass.Bass:
    global _PROGRAM
    if _PROGRAM is None:
        _PROGRAM = build_program()
    return _PROGRAM


def make_in_maps(x: np.ndarray, W_pred: np.ndarray, b_pred: np.ndarray):
    sel = np.ascontiguousarray(
        _build_sel().reshape(KH, NT * K * 128)).astype(ml_dtypes.bfloat16)
    # wb blob: [p, t*KH + k] = W_pred[t*128 + p, k]
    wb = (np.asarray(W_pred, dtype=np.float32)
          .reshape(NT, 128, KH).transpose(1, 0, 2).reshape(128, NT * KH)
          .astype(ml_dtypes.bfloat16))
    bias = np.asarray(b_pred, dtype=np.float32).reshape(KH, 1)
    ident = np.eye(128, dtype=ml_dtypes.bfloat16)
    in_maps = []
    for core in range(N_CORES):
        b_idx, hlf = divmod(core, 2)
        s0 = hlf * SHARD
        xp = np.zeros((SH, C), dtype=np.float32)
        lo = max(0, s0 - PAD)
        hi = min(S, s0 + SHARD + PAD)
        xp[lo - (s0 - PAD):hi - (s0 - PAD)] = x[b_idx, lo:hi]
        xt = np.ascontiguousarray(xp.T).astype(ml_dtypes.bfloat16)
        in_maps.append({"xt": xt, "wb": wb, "bias": bias, "sel": sel,
                        "ident": ident})
    return in_maps


def assemble(results) -> np.ndarray:
    out = np.empty((B, S, C), dtype=np.float32)
    for core in range(N_CORES):
        b_idx, hlf = divmod(core, 2)
        out[b_idx, hlf * SHARD:(hlf + 1) * SHARD] = (
            results[core]["out"].astype(np.float32).T
        )
    return out


def kernel(x: np.ndarray, W_pred: np.ndarray, b_pred: np.ndarray) -> np.ndarray:
    nc = _get_program()
    in_maps = make_in_maps(np.asarray(x), np.asarray(W_pred), np.asarray(b_pred))
    res = run_bass_kernel_spmd(nc, in_maps, list(range(N_CORES)))
    return assemble(res.results)


# revision 3
# speedup vs baseline: 1.1054x; 1.0458x over previous
"""DynamicConv1D Trainium2 kernel v2 (drain/direct split conv pipeline).

Reference computation (per batch b, position s):
    kern[s, h, i] = sum_c x[s, c] * W_pred[c, h*7+i] + b_pred[h*7+i]
    out[s, h, d]  = sum_i kern[s, h, i] * x_pad[s + i, h, d]     (pad = 3)

Sharding: 8 cores = (batch 4) x (sequence halves 2).  Each core gets the
transposed bf16 shard xt [1024, 1030] (1024 positions + 3-halo, zero-padded
at sequence ends) and produces out [1024c, 1024s] bf16.

Per-core pipeline (bf16 everywhere except PSUM):
  kern:  PE matmul (bf16 in, f32 PSUM) -> ACT drain (+bias) -> bf16 SBUF.
  conv per (ctile t, half h):  kb[c,s] = kern[head(c)*7+i, s] per tap i
    PE broadcast-fills kb taps into f32 PSUM pair tiles [128, 2, 512].
    taps 0-3: ACT drains pairs -> SBUF bf16; Pool multiplies vs shifted xt
              (gpsimd has no alignment-sensitive fast mode to lose).
    taps 4-6: DVE multiplies f32 PSUM directly (1x mode, saves the drain).
    tap-sum:  two SWDGE CCE accumulate-DMAs fold p0..p3 in place; the
              remaining three joins alternate DVE/Pool via FLEX.
  out: per-ctile DMA [128, 1024] bf16 -> HBM.
"""

import os
import sys

for _p in ("/opt/trn_rl_repo",):
    if _p not in sys.path and os.path.isdir(_p):
        sys.path.insert(0, _p)

import numpy as np
import ml_dtypes

import concourse.bass as bass
import concourse.mybir as mybir
from concourse import tile
from concourse.bass_utils import run_bass_kernel_spmd
from concourse.bass_types import AP

B = 4
S = 2048
C = 1024
H = 16
K = 7
HD = 64
PAD = K // 2
KH = K * H  # 112

N_CORES = 8
SHARD = S // 2          # positions per core = 1024
SH = SHARD + 2 * PAD    # 1030 cols of xt (position s lives at col s+3)
NT = C // 128           # 8 channel tiles
SW = 512                # free-dim tile (one PSUM bank of f32)
NS = SHARD // SW        # 2 position halves

F32 = mybir.dt.float32
BF16 = mybir.dt.bfloat16
IDENT = mybir.ActivationFunctionType.Identity
ADD = mybir.AluOpType.add


def _build_sel() -> np.ndarray:
    """sel[p, t, i, c] = 1 iff p == (2t + c//64)*7 + i  (p in [0,112))."""
    sel = np.zeros((KH, NT, K, 128), dtype=np.float32)
    for t in range(NT):
        for i in range(K):
            for hh in range(2):  # two heads per 128-channel tile
                p = (2 * t + hh) * K + i
                sel[p, t, i, 64 * hh:64 * (hh + 1)] = 1.0
    return sel


def _shift_view(base: AP, n: int, sw: int) -> AP:
    """[128, n, sw] view: (j, s) -> base[:, j + s] (consecutive tap shifts)."""
    return AP(tensor=base.tensor, offset=base.offset,
              ap=[[base.ap[0][0], 128], [1, n], [1, sw]])


def build_program() -> bass.Bass:
    nc = bass.Bass(trn_type="TRN2")

    xt_d = nc.dram_tensor("xt", [C, SH], BF16, kind="ExternalInput")
    wb_d = nc.dram_tensor("wb", [128, NT * KH], BF16, kind="ExternalInput")
    bias_d = nc.dram_tensor("bias", [KH, 1], F32, kind="ExternalInput")
    sel_d = nc.dram_tensor("sel", [KH, NT * K * 128], BF16, kind="ExternalInput")
    id_d = nc.dram_tensor("ident", [128, 128], BF16, kind="ExternalInput")
    out_d = nc.dram_tensor("out", [C, SHARD], BF16, kind="ExternalOutput")

    with tile.TileContext(nc) as tc:
        with (
            tc.tile_pool(name="xt", bufs=1) as xt_pool,
            tc.tile_pool(name="wgt", bufs=1) as w_pool,
            tc.tile_pool(name="sel", bufs=1) as sel_pool,
            tc.tile_pool(name="kern", bufs=1) as kern_pool,
            tc.tile_pool(name="kbd", bufs=3) as kbd_pool,
            tc.tile_pool(name="prod", bufs=3) as prod_pool,
            tc.tile_pool(name="stage", bufs=NT) as stage_pool,
        ):
            # ---- loads ----
            xt_sb = xt_pool.tile([128, NT, SH], BF16)
            xt_r = xt_d.ap().rearrange("(t p) s -> p t s", p=128)
            wb_sb = w_pool.tile([128, NT, KH], BF16)
            wb_r = wb_d.ap().rearrange("p (t k) -> p t k", t=NT)
            bias_sb = w_pool.tile([KH, 1], F32)
            id_sb = w_pool.tile([128, 128], BF16)
            sel_sb = sel_pool.tile([KH, NT, K, 128], BF16)
            sel_r = sel_d.ap().rearrange("p (t x) -> p t x", t=NT)

            SPL = PAD + SW + PAD + 2  # 520: cols for kern sj0 + conv h0
            # small tensors ride the ACT ring; xt has SP to itself
            nc.scalar.dma_start(wb_sb[:, :, :].rearrange("p t k -> p (t k)"),
                                wb_r[:, :, :].rearrange("p t k -> p (t k)"))
            for q0 in (0, 2, 4, 6):
                nc.sync.dma_start(xt_sb[:, q0:q0 + 2, 0:SPL],
                                  xt_r[:, q0:q0 + 2, 0:SPL])
            nc.scalar.dma_start(bias_sb[:, :], bias_d[:, :])
            nc.scalar.dma_start(id_sb[:, :], id_d[:, :])

            def sel_dma(t0, t1, eng=None):
                (eng or nc.sync).dma_start(
                    sel_sb[:, t0:t1, :, :].rearrange("p t i c -> p (t i c)"),
                    sel_r[:, t0:t1, :].rearrange("p t x -> p (t x)"),
                )

            sel_dma(0, 1, nc.scalar)
            for q0 in (0, 4):
                nc.sync.dma_start(xt_sb[:, q0:q0 + 4, SPL:SH],
                                  xt_r[:, q0:q0 + 4, SPL:SH])
            sel_dma(1, 3)
            sel_dma(3, 6)
            sel_dma(6, 8)

            kern_sb = kern_pool.tile([KH, SHARD], BF16)
            stages = []
            for _t in range(NT):
                stage_t = stage_pool.tile([128, NS, SW], BF16, tag="stage",
                                          name=f"stage_{_t}")
                stages.append(stage_t)

            # ---- conv ----
            with (
                tc.tile_pool(name="kb", bufs=3, space="PSUM") as kb_pool,
                tc.tile_pool(name="kb6p", bufs=1, space="PSUM") as kb6_pool,
                tc.tile_pool(name="accp", bufs=1, space="PSUM") as acc_pool,
            ):
                prev = []

                def kern_phase(sj):
                    kps = acc_pool.tile([128, SW], F32, tag="acc",
                                        name=f"kps_{sj}")
                    if sj == 0:
                        # load the ACT function table off the critical path
                        nc.scalar.activation(kern_sb[0:2, 0:2],
                                             wb_sb[0:2, 0, 0:2], IDENT)
                        # PE p-state warm-up: ~3us of tiny matmuls so the
                        # kern/fill stream runs at full clock (overwritten
                        # by the start=True kern matmul below).
                        for _w in range(16):
                            nc.tensor.matmul(kps[0:2, 0:2], wb_sb[0:2, 0, 0:2],
                                             wb_sb[0:2, 0, 0:2],
                                             start=True, stop=True)
                    for m in range(NT):
                        nc.tensor.matmul(
                            kps[0:KH, :], wb_sb[:, m, :],
                            xt_sb[:, m, PAD + SW * sj:PAD + SW * sj + SW],
                            start=(m == 0), stop=(m == NT - 1),
                        )
                    nc.scalar.activation(
                        kern_sb[:, SW * sj:SW * (sj + 1)], kps[0:KH, :],
                        IDENT, bias=bias_sb[:, :],
                    )

                def acc_flush(pv):
                    # PE accumulate (two halves behind) + DVE final join
                    t, h, At, p03t, p45t, p6t = pv
                    acc = acc_pool.tile([128, SW], F32, tag="acc")
                    if At is not None:
                        srcs = (At[:, 0, :], At[:, 1, :],
                                p45t[:, 0, :], p45t[:, 1, :])
                    else:
                        # tail halves: PE is idle, Pool's fold would be late
                        srcs = (p03t[:, 0, :], p03t[:, 1, :], p03t[:, 2, :],
                                p03t[:, 3, :], p45t[:, 0, :], p45t[:, 1, :])
                    for n, src in enumerate(srcs):
                        nc.tensor.matmul(acc[:, :], id_sb[:, :], src,
                                         start=(n == 0), stop=(n == len(srcs) - 1))
                    st = stages[t]
                    nc.vector.tensor_add(st[:, h, :], acc[:, :], p6t[:, :])
                    nc.sync.dma_start(
                        out_d[128 * t:128 * (t + 1), SW * h:SW * (h + 1)],
                        st[:, h, :])

                def half(t, h):
                    s0 = SW * h
                    # -- PE kb fills: pairs (4,5) (0,1) (2,3) + single 6 --
                    kb45 = kb_pool.tile([128, 2, SW], F32, tag="kb")
                    kb01 = kb_pool.tile([128, 2, SW], F32, tag="kb")
                    kb23 = kb_pool.tile([128, 2, SW], F32, tag="kb")
                    kb6 = kb6_pool.tile([128, SW], F32, tag="kb6")
                    for a, kb in ((0, kb01), (4, kb45), (2, kb23)):
                        for j in range(2):
                            nc.tensor.matmul(
                                kb[:, j, :], sel_sb[:, t, a + j, :],
                                kern_sb[:, s0:s0 + SW],
                                start=True, stop=True,
                            )
                    nc.tensor.matmul(kb6[:, :], sel_sb[:, t, 6, :],
                                     kern_sb[:, s0:s0 + SW],
                                     start=True, stop=True)
                    # -- taps 4-6: DVE direct from f32 PSUM --
                    p45 = prod_pool.tile([128, 2, SW], BF16, tag="p45")
                    nc.vector.tensor_mul(
                        p45[:, :, :], kb45[:, :, :],
                        _shift_view(xt_sb[:, t, s0 + 4:s0 + SW + 5], 2, SW))
                    p6 = prod_pool.tile([128, SW], BF16, tag="p6")
                    nc.vector.tensor_mul(
                        p6[:, :], kb6[:, :],
                        xt_sb[:, t, s0 + 6:s0 + 6 + SW])
                    # -- taps 0-3: ACT drains -> Pool product + fold --
                    kbd = kbd_pool.tile([128, 4, SW], BF16, tag="kbd")
                    nc.scalar.activation(kbd[:, 0:2, :], kb01[:, :, :], IDENT)
                    nc.scalar.activation(kbd[:, 2:4, :], kb23[:, :, :], IDENT)
                    p03 = prod_pool.tile([128, 4, SW], BF16, tag="p03")
                    if (t, h) == (0, 0):
                        # prefix: start multiplying after the first drain
                        nc.gpsimd.tensor_mul(
                            p03[:, 0:2, :], kbd[:, 0:2, :],
                            _shift_view(xt_sb[:, t, s0:s0 + SW + 1], 2, SW))
                        nc.gpsimd.tensor_mul(
                            p03[:, 2:4, :], kbd[:, 2:4, :],
                            _shift_view(xt_sb[:, t, s0 + 2:s0 + SW + 3], 2,
                                        SW))
                    else:
                        nc.gpsimd.tensor_mul(
                            p03[:, :, :], kbd[:, :, :],
                            _shift_view(xt_sb[:, t, s0:s0 + SW + 3], 4, SW))
                    if (h, t) in ((1, NT - 2), (1, NT - 1)):
                        At = None
                    else:
                        At = prod_pool.tile([128, 2, SW], BF16, tag="at")
                        nc.gpsimd.tensor_add(At[:, :, :], p03[:, 0:2, :],
                                             p03[:, 2:4, :])
                    # accumulate + final join, two halves behind (keeps
                    # PE fills from stalling on DVE/Pool of the last half)
                    prev.append((t, h, At, p03, p45, p6))
                    if len(prev) > 2:
                        acc_flush(prev.pop(0))

                kern_phase(0)
                half(0, 0)
                half(1, 0)
                kern_phase(1)
                for t in range(2, NT):
                    half(t, 0)
                for t in range(NT - 1):
                    half(t, 1)
                acc_flush(prev.pop(0))
                half(NT - 1, 1)
                for pv in prev:
                    acc_flush(pv)

    _strip_same_engine_waits(nc)
    return nc


# Engines complete their own instructions in program order (PE matmuls are
# pc-monotone in start AND end), so a wait on the engine's own completion
# semaphore is always satisfied by program order.  Tile still emits them for
# PSUM-slot WAW tracking; walrus then rejects matmuls with >1 wait (the
# LDWEIGHTS struct has a single sync-wait slot).  Strip them.
def _strip_same_engine_waits(nc: bass.Bass) -> None:
    # (1) PE matmuls complete in strict pc order (silicon), so a wait on PE's
    # own completion semaphore is redundant -> strip, keeping each matmul at
    # <=1 wait (walrus LDWEIGHTS has a single sync-wait slot).
    # (2) The exit Drain waits on every semaphore ever used, exceeding the
    # struct's wait capacity.  Input-DMA lane waits are covered transitively:
    # each compute engine waited on those lanes before its last instruction,
    # and the Drain still waits on every engine's final count.  Keep only
    # engine sems and the out-DMA lanes (nothing else observes those).
    out_lanes = set()
    for blk in nc.m.functions[0].blocks:
        for inst in blk.instructions:
            if inst.opcode != "DMACopy":
                continue
            dst = inst.outs[0]
            if getattr(dst, "memref", "").startswith("out"):
                for u in (inst.sync_info.on_update if inst.sync_info else []):
                    out_lanes.add(u.ant_name)
    noop_n = [0]
    for blk in nc.m.functions[0].blocks:
        for inst in blk.instructions:
            si = inst.sync_info
            if si is None or not si.on_wait:
                continue
            if str(inst.engine) == "EngineType.PE":
                kept = [w for w in si.on_wait if not w.ant_name.startswith("PE_")]
            elif inst.opcode == "Drain":
                kept = [
                    w for w in si.on_wait
                    if not w.ant_name.startswith("DMAHW") or w.ant_name in out_lanes
                ]
            else:
                continue
            if len(kept) != len(si.on_wait):
                inst.sync_info = mybir.SyncInfo(
                    on_wait=kept, on_update=list(si.on_update)
                )
    # Any instruction still carrying >1 wait: keep the first wait and move the
    # extras onto single-wait NoOps inserted just before it (same engine) --
    # the walrus instruction structs have a single sync-wait slot.
    for blk in nc.m.functions[0].blocks:
        il = blk.instructions
        idx = 0
        while idx < len(il):
            inst = il[idx]
            si = inst.sync_info
            if si is not None and len(si.on_wait) > 1:
                waits = list(si.on_wait)
                for w in waits[:-1]:
                    noop_n[0] += 1
                    nop = mybir.InstNoOp(
                        name=f"I-waitsplit-{noop_n[0]}",
                        engine=inst.engine,
                        ins=[], outs=[],
                        sync_info=mybir.SyncInfo(on_wait=[w], on_update=[]),
                    )
                    nc.register_instruction(nop, overwrite=True)
                    il.insert(idx, nop)
                    idx += 1
                inst.sync_info = mybir.SyncInfo(
                    on_wait=[waits[-1]], on_update=list(si.on_update)
                )
            idx += 1



_PROGRAM = None


def _get_program() -> bass.Bass:
    global _PROGRAM
    if _PROGRAM is None:
        _PROGRAM = build_program()
    return _PROGRAM


def make_in_maps(x: np.ndarray, W_pred: np.ndarray, b_pred: np.ndarray):
    sel = np.ascontiguousarray(
        _build_sel().reshape(KH, NT * K * 128)).astype(ml_dtypes.bfloat16)
    # wb blob: [p, t*KH + k] = W_pred[t*128 + p, k]
    wb = (np.asarray(W_pred, dtype=np.float32)
          .reshape(NT, 128, KH).transpose(1, 0, 2).reshape(128, NT * KH)
          .astype(ml_dtypes.bfloat16))
    bias = np.asarray(b_pred, dtype=np.float32).reshape(KH, 1)
    ident = np.eye(128, dtype=ml_dtypes.bfloat16)
    in_maps = []
    for core in range(N_CORES):
        b_idx, hlf = divmod(core, 2)
        s0 = hlf * SHARD
        xp = np.zeros((SH, C), dtype=np.float32)
        lo = max(0, s0 - PAD)
        hi = min(S, s0 + SHARD + PAD)
        xp[lo - (s0 - PAD):hi - (s0 - PAD)] = x[b_idx, lo:hi]
        xt = np.ascontiguousarray(xp.T).astype(ml_dtypes.bfloat16)
        in_maps.append({"xt": xt, "wb": wb, "bias": bias, "sel": sel,
                        "ident": ident})
    return in_maps


def assemble(results) -> np.ndarray:
    out = np.empty((B, S, C), dtype=np.float32)
    for core in range(N_CORES):
        b_idx, hlf = divmod(core, 2)
        out[b_idx, hlf * SHARD:(hlf + 1) * SHARD] = (
            results[core]["out"].astype(np.float32).T
        )
    return out


def kernel(x: np.ndarray, W_pred: np.ndarray, b_pred: np.ndarray) -> np.ndarray:
    nc = _get_program()
    in_maps = make_in_maps(np.asarray(x), np.asarray(W_pred), np.asarray(b_pred))
    res = run_bass_kernel_spmd(nc, in_maps, list(range(N_CORES)))
    return assemble(res.results)
